# revision 6
# baseline (speedup 1.0000x reference)
"""Trainium2 Bass kernel for nn_Block_56667798504032 (dense transformer block).

Sharding: 8 cores = 4 batches x 2 query-halves. Each core computes LN1+qkv
for its batch's 2048 tokens, causal attention + LN2 + MLP for its own 1024
query tokens. Single SPMD program; per-core behavior differs only via data
(token permutation + one exp-bias scalar).

Tricks:
- Both LayerNorms are folded into the following matmuls via an augmented
  contraction chunk (host-precomputed u/z weight rows; device-computed
  (-mu*rstd, 1) data rows). No normalized activations are materialized.
- No on-device transposes: host supplies x^T; scores are computed
  transposed (k on partitions) so softmax runs on the free axis; y is
  produced transposed (c on partitions), ready for fc; final output is
  produced transposed and un-transposed on host.
- Per-head softmax denominators are folded into the LN2 y-scaling pass.
- Causal masks: static affine_select on diagonal k-tiles; other-half
  k-tiles via a data-driven exp bias (0 or -1e4).
- float32r on the PE everywhere (bf16 speed at N>=256, ~1.6e-4 error).
- Phases 0-2 run per token-half so the scaled-x working set fits SBUF.
"""
import math
import numpy as np

import concourse.bass as bass
import concourse.mybir as mybir
import concourse.tile as tile
from concourse import bacc
from concourse.bass_utils import run_bass_kernel_spmd

F32 = mybir.dt.float32
F32R = mybir.dt.float32r
AF = mybir.ActivationFunctionType

FULL_DIMS = dict(B=4, T=2048, C=2048, NH=16, HD=128)
EPS = 1e-5
N_CORES = 8


def build_nc(dims):
    B, T, C, NH, HD = dims["B"], dims["T"], dims["C"], dims["NH"], dims["HD"]
    assert HD == 128
    TQ = T // 2
    HT = T // 2           # token half width (== TQ)
    CH = C // 128
    CHA = CH + 1
    M4 = 4 * C
    MCH = M4 // 128
    MCHA = MCH + 1
    QS = 512
    NQS = TQ // QS
    NKT = T // 128
    OWN_KT = TQ // 128
    DIAG = QS // 128
    NMT = M4 // 128
    NNT = C // 128
    PSUP = 16             # proj weight super-chunk (chunks per DMA)

    nc = bacc.Bacc(None, target_bir_lowering=False)
    with tile.TileContext(nc) as tc:
        with tc.tile_pool(name="dram", bufs=1, space="DRAM") as dram:
            xT = dram.tile([C, T], F32R, kind="ExternalInput", uniquify=False, name="xT")
            wq = dram.tile([CHA * 128, C], F32R, kind="ExternalInput", uniquify=False, name="wq")
            wk = dram.tile([CHA * 128, C], F32R, kind="ExternalInput", uniquify=False, name="wk")
            wv = dram.tile([CHA * 128, C], F32R, kind="ExternalInput", uniquify=False, name="wv")
            wfc = dram.tile([CHA * 128, M4], F32R, kind="ExternalInput", uniquify=False, name="wfc")
            wpr = dram.tile([MCHA * 128, C], F32R, kind="ExternalInput", uniquify=False, name="wpr")
            pbias = dram.tile([1, 1], F32, kind="ExternalInput", uniquify=False, name="pbias")
            outT = dram.tile([C, TQ], F32, kind="ExternalOutput", uniquify=False, name="outT")
            kd = dram.tile([NH, 128, T], F32R, name="kd")
            qd = dram.tile([NH, 128, TQ], F32R, name="qd")
            vdr = dram.tile([T // 128, 128, C], F32R, name="vdr")
            ysd = dram.tile([CHA, 128, TQ], F32R, name="ysd")

            xT_r = xT[:].rearrange("(ch p) t -> p ch t", p=128)
            wq_r = wq[:].rearrange("(ch p) n -> p ch n", p=128)
            wk_r = wk[:].rearrange("(ch p) n -> p ch n", p=128)
            wv_r = wv[:].rearrange("(ch p) n -> p ch n", p=128)
            wfc_r = wfc[:].rearrange("(ch p) n -> p ch n", p=128)
            wpr_r = wpr[:].rearrange("(ch p) n -> p ch n", p=128)
            outT_r = outT[:].rearrange("(nt p) t -> p nt t", p=128)

            # ====== Phases 0-2 per token half: LN1 stats, scale, qkv ======
            for hf in range(2):
                hsl = slice(hf * HT, (hf + 1) * HT)
                with tc.tile_pool(name=f"sb_xs{hf}", bufs=1) as sbxs:
                    xs = sbxs.tile([128, CHA, HT], F32R, name="xs")

                    with (
                        tc.tile_pool(name=f"p0_sb{hf}", bufs=1) as sb0,
                        tc.tile_pool(name=f"p0_st{hf}", bufs=3) as st0,
                        tc.tile_pool(name=f"p0_ps{hf}", bufs=2, space="PSUM") as ps0,
                    ):
                        ones0f = sb0.tile([128, 1], F32, name="ones0f")
                        nc.vector.memset(ones0f[:], 1.0)
                        ones0 = sb0.tile([128, 1], F32R, name="ones0")
                        nc.vector.tensor_copy(ones0[:], ones0f[:])
                        eps_t = sb0.tile([1, 1], F32, name="eps_t")
                        nc.vector.memset(eps_t[:], EPS)
                        sum_row = sb0.tile([1, HT], F32, name="sum_row")
                        sq_row = sb0.tile([1, HT], F32, name="sq_row")
                        rstd_row = sb0.tile([1, HT], F32, name="rstd_row")
                        tmp_row = sb0.tile([1, HT], F32, name="tmp_row")

                        # stats pass: stream xT chunks of this half
                        for sl in range(HT // 512):
                            ps_s = ps0.tile([1, 512], F32, name="ps_s", tag="ps_s")
                            ps_q = ps0.tile([1, 512], F32, name="ps_q", tag="ps_q")
                            csl = slice(hf * HT + sl * 512, hf * HT + (sl + 1) * 512)
                            for ch in range(CH):
                                xc = st0.tile([128, 512], F32R, name="xc", tag="xc")
                                nc.sync.dma_start(xc[:], xT_r[:, ch, csl])
                                nc.tensor.matmul(ps_s[:], ones0[:], xc[:],
                                                 start=(ch == 0), stop=(ch == CH - 1))
                                x2 = st0.tile([128, 512], F32R, name="x2", tag="x2")
                                nc.scalar.activation(x2[:], xc[:], AF.Square)
                                nc.tensor.matmul(ps_q[:], ones0[:], x2[:],
                                                 start=(ch == 0), stop=(ch == CH - 1))
                            nc.scalar.copy(sum_row[:, sl * 512:(sl + 1) * 512], ps_s[:])
                            nc.scalar.copy(sq_row[:, sl * 512:(sl + 1) * 512], ps_q[:])

                        nc.scalar.mul(sum_row[:], sum_row[:], 1.0 / C)        # mean
                        nc.scalar.mul(sq_row[:], sq_row[:], 1.0 / C)          # E[x2]
                        nc.vector.tensor_mul(tmp_row[:], sum_row[:], sum_row[:])
                        nc.vector.tensor_sub(sq_row[:], sq_row[:], tmp_row[:])  # var
                        nc.scalar.activation(rstd_row[:], sq_row[:], AF.Sqrt, bias=eps_t[:])
                        nc.vector.reciprocal(rstd_row[:], rstd_row[:])        # rstd
                        nc.vector.tensor_mul(tmp_row[:], sum_row[:], rstd_row[:])
                        nc.scalar.mul(tmp_row[:], tmp_row[:], -1.0)           # -mu*rstd

                        bc = sb0.tile([128, HT], F32R, name="bc")
                        nc.gpsimd.partition_broadcast(bc[:], rstd_row[:].bitcast(F32R))
                        # scale pass: re-stream xT chunks, write xs
                        for ch in range(CH):
                            xc2 = st0.tile([128, HT], F32R, name="xc2", tag="xc2")
                            nc.sync.dma_start(xc2[:], xT_r[:, ch, hsl])
                            nc.vector.tensor_mul(xs[:, ch, :], xc2[:], bc[:])
                        zf0 = sb0.tile([128, HT], F32, name="zf0")
                        nc.vector.memset(zf0[:], 0.0)
                        nc.vector.memset(zf0[0:2, :], 1.0)
                        nc.vector.tensor_copy(xs[:, CH, :], zf0[:])
                        nc.vector.tensor_copy(xs[0:1, CH, :], tmp_row[:])

                    # ---- qkv projections for this half
                    with (
                        tc.tile_pool(name=f"p2_w{hf}", bufs=3) as wp2,
                        tc.tile_pool(name=f"p2_wv{hf}", bufs=2) as wvp2,
                        tc.tile_pool(name=f"p2_ev{hf}", bufs=3) as evp2,
                        tc.tile_pool(name=f"p2_ps{hf}", bufs=1, space="PSUM") as psk,
                        tc.tile_pool(name=f"p2_psv{hf}", bufs=2, space="PSUM") as psv2,
                    ):
                        targets = [(wk_r, kd, hf * HT)]
                        if hf == 0:
                            targets.append((wq_r, qd, 0))
                        for (wr, dst, obase) in targets:
                            for ot in range(NNT):
                                wt = wp2.tile([128, CHA, 128], F32R, name="wt", tag="wblk")
                                nc.sync.dma_start(wt[:], wr[:, :, ot * 128:(ot + 1) * 128])
                                pss = [psk.tile([128, 512], F32, name=f"pk{i}", tag=f"pk{i}")
                                       for i in range(HT // 512)]
                                for ch in range(CHA):
                                    for sl in range(HT // 512):
                                        nc.tensor.matmul(
                                            pss[sl][:], wt[:, ch, :],
                                            xs[:, ch, sl * 512:(sl + 1) * 512],
                                            start=(ch == 0), stop=(ch == CHA - 1))
                                for sl in range(HT // 512):
                                    ev = evp2.tile([128, 512], F32R, name="ev", tag="ev")
                                    nc.scalar.copy(ev[:], pss[sl][:])
                                    nc.sync.dma_start(
                                        dst[ot, :, obase + sl * 512:obase + (sl + 1) * 512],
                                        ev[:])
                        for osl in range(C // 512):
                            wvt = wvp2.tile([128, CHA, 512], F32R, name="wvt", tag="wv")
                            nc.sync.dma_start(wvt[:], wv_r[:, :, osl * 512:(osl + 1) * 512])
                            for lt in range(HT // 128):
                                tt = hf * (HT // 128) + lt
                                psv = psv2.tile([128, 512], F32, name="psv", tag="psv")
                                for ch in range(CHA):
                                    nc.tensor.matmul(
                                        psv[:], xs[:, ch, lt * 128:(lt + 1) * 128],
                                        wvt[:, ch, :],
                                        start=(ch == 0), stop=(ch == CHA - 1))
                                ev = evp2.tile([128, 512], F32R, name="evv", tag="ev")
                                nc.scalar.copy(ev[:], psv[:])
                                nc.sync.dma_start(vdr[tt, :, osl * 512:(osl + 1) * 512], ev[:])

            # ============ Phase 3: causal attention ============
            with tc.tile_pool(name="p3_sb", bufs=1) as sb3:
                yT = sb3.tile([128, NH, TQ], F32R, name="yT")
                rd = sb3.tile([1, NH, TQ], F32, name="rd")
                ones1f = sb3.tile([128, 1], F32, name="ones1f")
                nc.vector.memset(ones1f[:], 1.0)
                ones1 = sb3.tile([128, 1], F32R, name="ones1")
                nc.vector.tensor_copy(ones1[:], ones1f[:])
                pbr = sb3.tile([1, 1], F32, name="pbr")
                nc.sync.dma_start(pbr[:], pbias[:])
                pb = sb3.tile([128, 1], F32, name="pb")
                nc.gpsimd.partition_broadcast(pb[:], pbr[:])

                with (
                    tc.tile_pool(name="p3_h", bufs=2) as hp3,
                    tc.tile_pool(name="p3_e", bufs=3) as ep3,
                    tc.tile_pool(name="p3_psy", bufs=2, space="PSUM") as psy3,
                    tc.tile_pool(name="p3_psd", bufs=2, space="PSUM") as psd3,
                    tc.tile_pool(name="p3_pss", bufs=2, space="PSUM") as pss3,
                ):
                    for h in range(NH):
                        kh = hp3.tile([128, T], F32R, name="kh", tag="kh")
                        nc.sync.dma_start(kh[:], kd[h, :, :])
                        qh = hp3.tile([128, TQ], F32R, name="qh", tag="qh")
                        nc.sync.dma_start(qh[:], qd[h, :, :])
                        vh = hp3.tile([128, T // 128, 128], F32R, name="vh", tag="vh")
                        nc.sync.dma_start(
                            vh[:],
                            vdr[:].rearrange("t p n -> p t n")[:, :, h * 128:(h + 1) * 128])

                        for s in range(NQS):
                            ktiles = (list(range(0, DIAG * s + DIAG))
                                      + list(range(OWN_KT, NKT)))
                            psy = psy3.tile([128, QS], F32, name="psy", tag="psy")
                            psd = psd3.tile([1, QS], F32, name="psd", tag="psd")
                            for idx, j in enumerate(ktiles):
                                pss = pss3.tile([128, QS], F32, name="pss", tag="pss")
                                nc.tensor.matmul(pss[:], kh[:, j * 128:(j + 1) * 128],
                                                 qh[:, s * QS:(s + 1) * QS],
                                                 start=True, stop=True)
                                es = ep3.tile([128, QS], F32R, name="es", tag="es")
                                if j >= OWN_KT:
                                    nc.scalar.activation(es[:], pss[:], AF.Exp, bias=pb[:])
                                else:
                                    nc.scalar.activation(es[:], pss[:], AF.Exp)
                                if DIAG * s <= j < DIAG * s + DIAG:
                                    d = j - DIAG * s
                                    nc.gpsimd.affine_select(
                                        es[:], es[:], [[1, QS]], mybir.AluOpType.is_ge,
                                        0.0, base=-128 * d, channel_multiplier=-1)
                                first, last = (idx == 0), (idx == len(ktiles) - 1)
                                nc.tensor.matmul(psy[:], vh[:, j, :], es[:],
                                                 start=first, stop=last)
                                nc.tensor.matmul(psd[:], ones1[:], es[:],
                                                 start=first, stop=last)
                            nc.scalar.copy(yT[:, h, s * QS:(s + 1) * QS], psy[:])
                            nc.scalar.copy(rd[:, h, s * QS:(s + 1) * QS], psd[:])

                # ============ Phase 4/5: LN2 stats + y scaling ============
                with (
                    tc.tile_pool(name="p45_sb", bufs=1) as sb45,
                    tc.tile_pool(name="p45_w", bufs=2) as w45,
                    tc.tile_pool(name="p45_t", bufs=3) as t45,
                    tc.tile_pool(name="p45_ps", bufs=2, space="PSUM") as ps45,
                ):
                    rdf = rd[:].rearrange("o nh t -> o (nh t)")
                    nc.vector.reciprocal(rdf, rdf)
                    m_row = sb45.tile([1, TQ], F32, name="m_row")
                    s_row = sb45.tile([1, TQ], F32, name="s_row")
                    t_row = sb45.tile([1, TQ], F32, name="t_row")
                    r2_row = sb45.tile([1, TQ], F32, name="r2_row")
                    nm2_row = sb45.tile([1, TQ], F32, name="nm2_row")
                    eps2_t = sb45.tile([1, 1], F32, name="eps2_t")
                    nc.vector.memset(eps2_t[:], EPS)
                    nc.vector.memset(m_row[:], 0.0)
                    nc.vector.memset(s_row[:], 0.0)
                    for h in range(NH):
                        y2 = w45.tile([128, TQ], F32R, name="y2", tag="y2")
                        nc.scalar.activation(y2[:], yT[:, h, :], AF.Square)
                        for s in range(NQS):
                            qsl = slice(s * QS, (s + 1) * QS)
                            ps_m = ps45.tile([1, QS], F32, name="ps_m", tag="ps_m")
                            ps_q2 = ps45.tile([1, QS], F32, name="ps_q2", tag="ps_q2")
                            nc.tensor.matmul(ps_m[:], ones1[:], yT[:, h, qsl],
                                             start=True, stop=True)
                            nc.tensor.matmul(ps_q2[:], ones1[:], y2[:, qsl],
                                             start=True, stop=True)
                            tr = t45.tile([1, QS], F32, name="tr", tag="tr")
                            nc.vector.tensor_mul(tr[:], ps_m[:], rd[:, h, qsl])
                            nc.vector.tensor_add(m_row[:, qsl], m_row[:, qsl], tr[:])
                            nc.vector.tensor_mul(tr[:], ps_q2[:], rd[:, h, qsl])
                            nc.vector.tensor_mul(tr[:], tr[:], rd[:, h, qsl])
                            nc.vector.tensor_add(s_row[:, qsl], s_row[:, qsl], tr[:])
                    nc.scalar.mul(m_row[:], m_row[:], 1.0 / C)
                    nc.scalar.mul(s_row[:], s_row[:], 1.0 / C)
                    nc.vector.tensor_mul(t_row[:], m_row[:], m_row[:])
                    nc.vector.tensor_sub(s_row[:], s_row[:], t_row[:])
                    nc.scalar.activation(r2_row[:], s_row[:], AF.Sqrt, bias=eps2_t[:])
                    nc.vector.reciprocal(r2_row[:], r2_row[:])
                    nc.vector.tensor_mul(nm2_row[:], m_row[:], r2_row[:])
                    nc.scalar.mul(nm2_row[:], nm2_row[:], -1.0)

                    for h in range(NH):
                        cr = t45.tile([1, TQ], F32, name="cr", tag="cr")
                        nc.vector.tensor_mul(cr[:], r2_row[:], rd[:, h, :])
                        bch = w45.tile([128, TQ], F32R, name="bch", tag="bch")
                        nc.gpsimd.partition_broadcast(bch[:], cr[:].bitcast(F32R))
                        ys = w45.tile([128, TQ], F32R, name="ys", tag="ys")
                        nc.vector.tensor_mul(ys[:], yT[:, h, :], bch[:])
                        nc.sync.dma_start(ysd[h, :, :], ys[:])
                    ysa = w45.tile([128, TQ], F32R, name="ysa", tag="ys")
                    zf45 = w45.tile([128, TQ], F32, name="zf45", tag="zf45")
                    nc.vector.memset(zf45[:], 0.0)
                    nc.vector.memset(zf45[0:2, :], 1.0)
                    nc.vector.tensor_copy(ysa[:], zf45[:])
                    nc.vector.tensor_copy(ysa[0:1, :], nm2_row[:])
                    nc.sync.dma_start(ysd[CH, :, :], ysa[:])

            # ============ Phase 6: MLP ============
            ysd_r = ysd[:].rearrange("ch p t -> p ch t")
            for ts in range(NQS):
                with tc.tile_pool(name=f"p6_act{ts}", bufs=1) as sb6:
                    act = sb6.tile([128, MCHA, QS], F32R, name="act")
                    zf6 = sb6.tile([128, QS], F32, name="zf6")
                    nc.vector.memset(zf6[:], 0.0)
                    nc.vector.memset(zf6[0:1, :], 1.0)
                    nc.vector.tensor_copy(act[:, MCH, :], zf6[:])
                    with (
                        tc.tile_pool(name=f"p6f_sb{ts}", bufs=1) as sbf,
                        tc.tile_pool(name=f"p6f_w{ts}", bufs=3) as wf6,
                        tc.tile_pool(name=f"p6f_ps{ts}", bufs=3, space="PSUM") as psf6,
                    ):
                        ysl = sbf.tile([128, CHA, QS], F32R, name="ysl")
                        nc.sync.dma_start(ysl[:], ysd_r[:, :, ts * QS:(ts + 1) * QS])
                        for mt in range(NMT):
                            wt = wf6.tile([128, CHA, 128], F32R, name="wt6", tag="w6")
                            nc.sync.dma_start(wt[:], wfc_r[:, :, mt * 128:(mt + 1) * 128])
                            psf = psf6.tile([128, QS], F32, name="psf", tag="psf")
                            for ch in range(CHA):
                                nc.tensor.matmul(psf[:], wt[:, ch, :], ysl[:, ch, :],
                                                 start=(ch == 0), stop=(ch == CHA - 1))
                            nc.scalar.activation(act[:, mt, :], psf[:], AF.Gelu)
                    with (
                        tc.tile_pool(name=f"p6p_w{ts}", bufs=3) as wp6,
                        tc.tile_pool(name=f"p6p_ev{ts}", bufs=3) as evp6,
                        tc.tile_pool(name=f"p6p_ps{ts}", bufs=3, space="PSUM") as psp6,
                    ):
                        nsup = -(-MCHA // PSUP)
                        for nt in range(NNT):
                            pso = psp6.tile([128, QS], F32, name="pso", tag="pso")
                            for sp in range(nsup):
                                c0 = sp * PSUP
                                c1 = min(MCHA, c0 + PSUP)
                                wp = wp6.tile([128, PSUP, 128], F32R, name="wp6", tag="wp6")
                                nc.sync.dma_start(
                                    wp[:, 0:c1 - c0, :],
                                    wpr_r[:, c0:c1, nt * 128:(nt + 1) * 128])
                                for ch in range(c0, c1):
                                    nc.tensor.matmul(pso[:], wp[:, ch - c0, :],
                                                     act[:, ch, :],
                                                     start=(ch == 0), stop=(ch == MCHA - 1))
                            ev = evp6.tile([128, QS], F32, name="evo", tag="evo")
                            nc.scalar.copy(ev[:], pso[:])
                            nc.sync.dma_start(outT_r[:, nt, ts * QS:(ts + 1) * QS], ev[:])

    nc.compile()
    return nc


# ============ host side ============
_NC_CACHE = {}


def _get_nc(dims):
    key = tuple(sorted(dims.items()))
    if key not in _NC_CACHE:
        _NC_CACHE[key] = build_nc(dims)
    return _NC_CACHE[key]


def prep_weights(dims, ln1_w, ln1_b, attn_w, attn_b, ln2_w, ln2_b, fc_w, fc_b,
                 proj_w, proj_b):
    C = dims["C"]
    M4 = 4 * C
    CHA = C // 128 + 1
    MCHA = M4 // 128 + 1
    smscale = np.float32(1.0 / math.sqrt(dims["HD"]))

    def aug(wpart, bpart, g, bvec, scale=1.0):
        ncols = wpart.shape[1]
        out = np.zeros((CHA * 128, ncols), np.float32)
        wt = (g[:, None] * wpart).astype(np.float32)
        out[:C] = wt
        out[C] = wt.sum(0)
        out[C + 1] = bvec @ wpart + bpart
        return np.ascontiguousarray(out * np.float32(scale))

    wq = aug(attn_w[:, 0:C], attn_b[0:C], ln1_w, ln1_b, smscale)
    wk = aug(attn_w[:, C:2 * C], attn_b[C:2 * C], ln1_w, ln1_b)
    wv = aug(attn_w[:, 2 * C:3 * C], attn_b[2 * C:3 * C], ln1_w, ln1_b)
    wfc = aug(fc_w, fc_b, ln2_w, ln2_b)
    wpr = np.zeros((MCHA * 128, C), np.float32)
    wpr[:M4] = proj_w
    wpr[M4] = proj_b
    return wq, wk, wv, wfc, np.ascontiguousarray(wpr)


def kernel(x, ln1_w, ln1_b, attn_w, attn_b, ln2_w, ln2_b, fc_w, fc_b, proj_w,
           proj_b, dims=None, n_cores=None, trace=False):
    dims = dims or FULL_DIMS
    n_cores = n_cores if n_cores is not None else N_CORES
    B, T, C = dims["B"], dims["T"], dims["C"]
    TQ = T // 2
    x = np.asarray(x, np.float32)
    args = [np.asarray(a, np.float32) for a in
            (ln1_w, ln1_b, attn_w, attn_b, ln2_w, ln2_b, fc_w, fc_b, proj_w, proj_b)]
    wq, wk, wv, wfc, wpr = prep_weights(dims, *args)
    nc = _get_nc(dims)

    in_maps = []
    for c in range(n_cores):
        b, p = c // 2, c % 2
        xt = np.ascontiguousarray(x[b].T)
        if p == 1:
            xt = np.ascontiguousarray(np.concatenate([xt[:, TQ:], xt[:, :TQ]], axis=1))
        in_maps.append({
            "xT": xt, "wq": wq, "wk": wk, "wv": wv, "wfc": wfc, "wpr": wpr,
            "pbias": np.array([[0.0 if p == 1 else -10000.0]], np.float32),
        })

    res = run_bass_kernel_spmd(nc, in_maps, core_ids=list(range(n_cores)), trace=trace)
    out = np.empty((B, T, C), np.float32)
    for c in range(n_cores):
        b, p = c // 2, c % 2
        out[b, p * TQ:(p + 1) * TQ, :] = res.results[c]["outT"].T
    if trace:
        return out, res
    return out


# revision 10
# speedup vs baseline: 1.4417x; 1.4417x over previous
"""Trainium2 Bass kernel for nn_Block_56667798504032 (dense transformer block).

Sharding: 8 cores = 4 batches x 2 query-halves. Each core computes LN1+qkv
for its batch's 2048 tokens, causal attention + LN2 + MLP for its own 1024
query tokens. Single SPMD program; per-core behavior differs only via data
(token permutation + one exp-bias scalar).

Tricks:
- Both LayerNorms are folded into the following matmuls via an augmented
  contraction chunk (host-precomputed u/z weight rows; device-computed
  (-mu*rstd, 1) data rows). No normalized activations are materialized.
- No on-device transposes: host supplies x^T; scores are computed
  transposed (k on partitions) so softmax runs on the free axis; y is
  produced transposed (c on partitions), ready for fc; final output is
  produced transposed and un-transposed on host.
- Per-head softmax denominators: computed via a ones-column matmul and
  divided out during the attention PSUM eviction (reciprocal+broadcast
  overlap under the PE's next tile).
- Causal masks: static affine_select on diagonal k-tiles; other-half
  k-tiles via a data-driven exp bias (0 or -1e4).
- fp16 matmul operands (1 cycle/row like bf16, 8x the mantissa of bf16;
  fp32 PSUM accumulate). LayerNorm statistics in fp32.
"""
import math
import numpy as np

import concourse.bass as bass
import concourse.mybir as mybir
import concourse.tile as tile
from concourse import bacc
from concourse.bass_utils import run_bass_kernel_spmd

F32 = mybir.dt.float32
F32R = mybir.dt.float32r
BF16 = mybir.dt.bfloat16
FP16 = mybir.dt.float16
AF = mybir.ActivationFunctionType
DTYPES = {"f32r": F32R, "bf16": BF16, "fp16": FP16}

FULL_DIMS = dict(B=4, T=2048, C=2048, NH=16, HD=128)
EPS = 1e-5
N_CORES = 8


def build_nc(dims, dtype="fp16"):
    DT = DTYPES[dtype]
    B, T, C, NH, HD = dims["B"], dims["T"], dims["C"], dims["NH"], dims["HD"]
    assert HD == 128
    TQ = T // 2
    CH = C // 128
    CHA = CH + 1
    M4 = 4 * C
    MCH = M4 // 128
    MCHA = MCH + 1
    QS = 512
    NQS = TQ // QS
    NKT = T // 128
    OWN_KT = TQ // 128
    DIAG = QS // 128
    NMT = M4 // 128
    NNT = C // 128
    PSUP = 16

    nc = bacc.Bacc(None, target_bir_lowering=False)
    with tile.TileContext(nc) as tc:
        with tc.tile_pool(name="dram", bufs=1, space="DRAM") as dram:
            xT = dram.tile([C, T], DT, kind="ExternalInput", uniquify=False, name="xT")
            wq = dram.tile([CHA * 128, C], DT, kind="ExternalInput", uniquify=False, name="wq")
            wk = dram.tile([CHA * 128, C], DT, kind="ExternalInput", uniquify=False, name="wk")
            wv = dram.tile([CHA * 128, C], DT, kind="ExternalInput", uniquify=False, name="wv")
            wfc = dram.tile([CHA * 128, M4], DT, kind="ExternalInput", uniquify=False, name="wfc")
            wpr = dram.tile([MCHA * 128, C], DT, kind="ExternalInput", uniquify=False, name="wpr")
            pbias = dram.tile([1, 1], F32, kind="ExternalInput", uniquify=False, name="pbias")
            outT = dram.tile([C, TQ], F32, kind="ExternalOutput", uniquify=False, name="outT")
            kd = dram.tile([NH, 128, T], DT, name="kd")
            qd = dram.tile([NH, 128, TQ], DT, name="qd")
            vdr = dram.tile([T // 128, 128, C], DT, name="vdr")

            xT_r = xT[:].rearrange("(ch p) t -> p ch t", p=128)
            wq_r = wq[:].rearrange("(ch p) n -> p ch n", p=128)
            wk_r = wk[:].rearrange("(ch p) n -> p ch n", p=128)
            wv_r = wv[:].rearrange("(ch p) n -> p ch n", p=128)
            wfc_r = wfc[:].rearrange("(ch p) n -> p ch n", p=128)
            wpr_r = wpr[:].rearrange("(ch p) n -> p ch n", p=128)
            outT_r = outT[:].rearrange("(nt p) t -> p nt t", p=128)

            # ====== Phases 0-2: LN1 stats, scaled Xs, qkv projections ======
            with tc.tile_pool(name="sb_xs", bufs=1) as sbxs:
                xs = sbxs.tile([128, CHA, T], DT, name="xs")

                with (
                    tc.tile_pool(name="p0_sb", bufs=1) as sb0,
                    tc.tile_pool(name="p0_st", bufs=4) as st0,
                    tc.tile_pool(name="p0_ps", bufs=2, space="PSUM") as ps0,
                ):
                    ones0f = sb0.tile([128, 1], F32, name="ones0f")
                    nc.vector.memset(ones0f[:], 1.0)
                    ones0 = sb0.tile([128, 1], DT, name="ones0")
                    nc.vector.tensor_copy(ones0[:], ones0f[:])
                    eps_t = sb0.tile([1, 1], F32, name="eps_t")
                    nc.vector.memset(eps_t[:], EPS)
                    sum_row = sb0.tile([1, T], F32, name="sum_row")
                    sq_row = sb0.tile([1, T], F32, name="sq_row")
                    rstd_row = sb0.tile([1, T], F32, name="rstd_row")
                    tmp_row = sb0.tile([1, T], F32, name="tmp_row")

                    for sl in range(T // 512):
                        ps_s = ps0.tile([1, 512], F32, name="ps_s", tag="ps_s")
                        ps_q = ps0.tile([1, 512], F32, name="ps_q", tag="ps_q")
                        csl = slice(sl * 512, (sl + 1) * 512)
                        for ch in range(CH):
                            xc = st0.tile([128, 512], DT, name="xc", tag="xc")
                            nc.sync.dma_start(xc[:], xT_r[:, ch, csl])
                            nc.tensor.matmul(ps_s[:], ones0[:], xc[:],
                                             start=(ch == 0), stop=(ch == CH - 1))
                            x2 = st0.tile([128, 512], DT, name="x2", tag="x2")
                            nc.scalar.activation(x2[:], xc[:], AF.Square)
                            nc.tensor.matmul(ps_q[:], ones0[:], x2[:],
                                             start=(ch == 0), stop=(ch == CH - 1))
                        nc.scalar.copy(sum_row[:, csl], ps_s[:])
                        nc.scalar.copy(sq_row[:, csl], ps_q[:])

                    nc.scalar.mul(sum_row[:], sum_row[:], 1.0 / C)        # mean
                    nc.scalar.mul(sq_row[:], sq_row[:], 1.0 / C)          # E[x2]
                    nc.vector.tensor_mul(tmp_row[:], sum_row[:], sum_row[:])
                    nc.vector.tensor_sub(sq_row[:], sq_row[:], tmp_row[:])  # var
                    nc.scalar.activation(rstd_row[:], sq_row[:], AF.Sqrt, bias=eps_t[:])
                    nc.vector.reciprocal(rstd_row[:], rstd_row[:])        # rstd
                    nc.vector.tensor_mul(tmp_row[:], sum_row[:], rstd_row[:])
                    nc.scalar.mul(tmp_row[:], tmp_row[:], -1.0)           # -mu*rstd

                    bc = sb0.tile([128, T], DT, name="bc")
                    rstd_dt = sb0.tile([1, T], DT, name="rstd_dt")
                    nc.vector.tensor_copy(rstd_dt[:], rstd_row[:])
                    nc.gpsimd.partition_broadcast(bc[:], rstd_dt[:])
                    for ch in range(CH):
                        xc2 = st0.tile([128, T], DT, name="xc2", tag="xc2")
                        nc.sync.dma_start(xc2[:], xT_r[:, ch, :])
                        nc.vector.tensor_mul(xs[:, ch, :], xc2[:], bc[:])
                    zf0 = sb0.tile([128, T], F32, name="zf0")
                    nc.vector.memset(zf0[:], 0.0)
                    nc.vector.memset(zf0[0:2, :], 1.0)
                    nc.vector.tensor_copy(xs[:, CH, :], zf0[:])
                    nc.vector.tensor_copy(xs[0:1, CH, :], tmp_row[:])

                with (
                    tc.tile_pool(name="p2_w", bufs=3) as wp2,
                    tc.tile_pool(name="p2_wv", bufs=2) as wvp2,
                    tc.tile_pool(name="p2_ev", bufs=4) as evp2,
                    tc.tile_pool(name="p2_ps", bufs=2, space="PSUM") as psk,
                    tc.tile_pool(name="p2_psv", bufs=3, space="PSUM") as psv2,
                ):
                    for (wr, nsl, dst) in ((wk_r, T // 512, kd), (wq_r, TQ // 512, qd)):
                        for ot in range(NNT):
                            wt = wp2.tile([128, CHA, 128], DT, name="wt", tag="wblk")
                            nc.sync.dma_start(wt[:], wr[:, :, ot * 128:(ot + 1) * 128])
                            for sl in range(nsl):
                                ps = psk.tile([128, 512], F32, name="pk", tag="pk")
                                for ch in range(CHA):
                                    nc.tensor.matmul(
                                        ps[:], wt[:, ch, :],
                                        xs[:, ch, sl * 512:(sl + 1) * 512],
                                        start=(ch == 0), stop=(ch == CHA - 1))
                                ev = evp2.tile([128, 512], DT, name="ev", tag="ev")
                                nc.scalar.copy(ev[:], ps[:])
                                nc.sync.dma_start(dst[ot, :, sl * 512:(sl + 1) * 512], ev[:])
                    for osl in range(C // 512):
                        wvt = wvp2.tile([128, CHA, 512], DT, name="wvt", tag="wv")
                        nc.sync.dma_start(wvt[:], wv_r[:, :, osl * 512:(osl + 1) * 512])
                        for tt in range(T // 128):
                            psv = psv2.tile([128, 512], F32, name="psv", tag="psv")
                            for ch in range(CHA):
                                nc.tensor.matmul(
                                    psv[:], xs[:, ch, tt * 128:(tt + 1) * 128],
                                    wvt[:, ch, :],
                                    start=(ch == 0), stop=(ch == CHA - 1))
                            ev = evp2.tile([128, 512], DT, name="evv", tag="ev")
                            nc.scalar.copy(ev[:], psv[:])
                            nc.sync.dma_start(vdr[tt, :, osl * 512:(osl + 1) * 512], ev[:])

            # ============ Phase 3: causal attention (normalized inline) ====
            with tc.tile_pool(name="p3_sb", bufs=1) as sb3:
                yT = sb3.tile([128, NH, TQ], DT, name="yT")
                ysr = sb3.tile([128, CHA, TQ], DT, name="ysr")
                ones1f = sb3.tile([128, 1], F32, name="ones1f")
                nc.vector.memset(ones1f[:], 1.0)
                ones1 = sb3.tile([128, 1], DT, name="ones1")
                nc.vector.tensor_copy(ones1[:], ones1f[:])
                pbr = sb3.tile([1, 1], F32, name="pbr")
                nc.sync.dma_start(pbr[:], pbias[:])
                pb = sb3.tile([128, 1], F32, name="pb")
                nc.gpsimd.partition_broadcast(pb[:], pbr[:])

                with (
                    tc.tile_pool(name="p3_h", bufs=2) as hp3,
                    tc.tile_pool(name="p3_e", bufs=4) as ep3,
                    tc.tile_pool(name="p3_r", bufs=3) as rp3,
                    tc.tile_pool(name="p3_psy", bufs=2, space="PSUM") as psy3,
                    tc.tile_pool(name="p3_psd", bufs=2, space="PSUM") as psd3,
                    tc.tile_pool(name="p3_pss", bufs=3, space="PSUM") as pss3,
                ):
                    for h in range(NH):
                        kh = hp3.tile([128, T], DT, name="kh", tag="kh")
                        nc.sync.dma_start(kh[:], kd[h, :, :])
                        qh = hp3.tile([128, TQ], DT, name="qh", tag="qh")
                        nc.sync.dma_start(qh[:], qd[h, :, :])
                        vh = hp3.tile([128, T // 128, 128], DT, name="vh", tag="vh")
                        nc.sync.dma_start(
                            vh[:],
                            vdr[:].rearrange("t p n -> p t n")[:, :, h * 128:(h + 1) * 128])

                        for s in range(NQS):
                            ktiles = (list(range(0, DIAG * s + DIAG))
                                      + list(range(OWN_KT, NKT)))
                            psy = psy3.tile([128, QS], F32, name="psy", tag="psy")
                            psd = psd3.tile([1, QS], F32, name="psd", tag="psd")
                            for idx, j in enumerate(ktiles):
                                pss = pss3.tile([128, QS], F32, name="pss", tag="pss")
                                nc.tensor.matmul(pss[:], kh[:, j * 128:(j + 1) * 128],
                                                 qh[:, s * QS:(s + 1) * QS],
                                                 start=True, stop=True)
                                es = ep3.tile([128, QS], DT, name="es", tag="es")
                                if j >= OWN_KT:
                                    nc.scalar.activation(es[:], pss[:], AF.Exp, bias=pb[:])
                                else:
                                    nc.scalar.activation(es[:], pss[:], AF.Exp)
                                if DIAG * s <= j < DIAG * s + DIAG:
                                    d = j - DIAG * s
                                    nc.gpsimd.affine_select(
                                        es[:], es[:], [[1, QS]], mybir.AluOpType.is_ge,
                                        0.0, base=-128 * d, channel_multiplier=-1)
                                first, last = (idx == 0), (idx == len(ktiles) - 1)
                                nc.tensor.matmul(psy[:], vh[:, j, :], es[:],
                                                 start=first, stop=last)
                                nc.tensor.matmul(psd[:], ones1[:], es[:],
                                                 start=first, stop=last)
                            # normalize by softmax denominator during eviction
                            rdr = rp3.tile([1, QS], F32, name="rdr", tag="rdr")
                            nc.vector.reciprocal(rdr[:], psd[:])
                            bcd = rp3.tile([128, QS], F32, name="bcd", tag="bcd")
                            nc.gpsimd.partition_broadcast(bcd[:], rdr[:])
                            nc.vector.tensor_mul(yT[:, h, s * QS:(s + 1) * QS],
                                                 psy[:], bcd[:])

                # ============ Phase 4/5: LN2 stats + y scaling ============
                with (
                    tc.tile_pool(name="p45_sb", bufs=1) as sb45,
                    tc.tile_pool(name="p45_w", bufs=2) as w45,
                    tc.tile_pool(name="p45_ps", bufs=2, space="PSUM") as ps45,
                ):
                    m_row = sb45.tile([1, TQ], F32, name="m_row")
                    s_row = sb45.tile([1, TQ], F32, name="s_row")
                    t_row = sb45.tile([1, TQ], F32, name="t_row")
                    r2_row = sb45.tile([1, TQ], F32, name="r2_row")
                    nm2_row = sb45.tile([1, TQ], F32, name="nm2_row")
                    eps2_t = sb45.tile([1, 1], F32, name="eps2_t")
                    nc.vector.memset(eps2_t[:], EPS)
                    for s in range(NQS):
                        qsl = slice(s * QS, (s + 1) * QS)
                        ps_m = ps45.tile([1, QS], F32, name="ps_m", tag="ps_m")
                        ps_q2 = ps45.tile([1, QS], F32, name="ps_q2", tag="ps_q2")
                        for h in range(NH):
                            y2 = w45.tile([128, QS], DT, name="y2", tag="y2")
                            nc.scalar.activation(y2[:], yT[:, h, qsl], AF.Square)
                            nc.tensor.matmul(ps_m[:], ones1[:], yT[:, h, qsl],
                                             start=(h == 0), stop=(h == NH - 1))
                            nc.tensor.matmul(ps_q2[:], ones1[:], y2[:],
                                             start=(h == 0), stop=(h == NH - 1))
                        nc.scalar.copy(m_row[:, qsl], ps_m[:])
                        nc.scalar.copy(s_row[:, qsl], ps_q2[:])
                    nc.scalar.mul(m_row[:], m_row[:], 1.0 / C)
                    nc.scalar.mul(s_row[:], s_row[:], 1.0 / C)
                    nc.vector.tensor_mul(t_row[:], m_row[:], m_row[:])
                    nc.vector.tensor_sub(s_row[:], s_row[:], t_row[:])
                    nc.scalar.activation(r2_row[:], s_row[:], AF.Sqrt, bias=eps2_t[:])
                    nc.vector.reciprocal(r2_row[:], r2_row[:])
                    nc.vector.tensor_mul(nm2_row[:], m_row[:], r2_row[:])
                    nc.scalar.mul(nm2_row[:], nm2_row[:], -1.0)

                    r2d = sb45.tile([1, TQ], DT, name="r2d")
                    nc.vector.tensor_copy(r2d[:], r2_row[:])
                    bc2 = sb45.tile([128, TQ], DT, name="bc2")
                    nc.gpsimd.partition_broadcast(bc2[:], r2d[:])
                    for h in range(NH):
                        nc.vector.tensor_mul(ysr[:, h, :], yT[:, h, :], bc2[:])
                    zf45 = sb45.tile([128, TQ], F32, name="zf45")
                    nc.vector.memset(zf45[:], 0.0)
                    nc.vector.memset(zf45[0:2, :], 1.0)
                    nc.vector.tensor_copy(ysr[:, CH, :], zf45[:])
                    nc.vector.tensor_copy(ysr[0:1, CH, :], nm2_row[:])

                # ============ Phase 6: MLP ============
                for ts in range(NQS):
                    with tc.tile_pool(name=f"p6_act{ts}", bufs=1) as sb6:
                        act = sb6.tile([128, MCHA, QS], DT, name="act")
                        zf6 = sb6.tile([128, QS], F32, name="zf6")
                        nc.vector.memset(zf6[:], 0.0)
                        nc.vector.memset(zf6[0:1, :], 1.0)
                        nc.vector.tensor_copy(act[:, MCH, :], zf6[:])
                        with (
                            tc.tile_pool(name=f"p6f_w{ts}", bufs=3) as wf6,
                            tc.tile_pool(name=f"p6f_ps{ts}", bufs=3, space="PSUM") as psf6,
                        ):
                            for mt in range(NMT):
                                wt = wf6.tile([128, CHA, 128], DT, name="wt6", tag="w6")
                                nc.sync.dma_start(wt[:], wfc_r[:, :, mt * 128:(mt + 1) * 128])
                                psf = psf6.tile([128, QS], F32, name="psf", tag="psf")
                                for ch in range(CHA):
                                    nc.tensor.matmul(
                                        psf[:], wt[:, ch, :],
                                        ysr[:, ch, ts * QS:(ts + 1) * QS],
                                        start=(ch == 0), stop=(ch == CHA - 1))
                                nc.scalar.activation(act[:, mt, :], psf[:], AF.Gelu)
                        with (
                            tc.tile_pool(name=f"p6p_w{ts}", bufs=3) as wp6,
                            tc.tile_pool(name=f"p6p_ev{ts}", bufs=3) as evp6,
                            tc.tile_pool(name=f"p6p_ps{ts}", bufs=3, space="PSUM") as psp6,
                        ):
                            nsup = -(-MCHA // PSUP)
                            for nt in range(NNT):
                                pso = psp6.tile([128, QS], F32, name="pso", tag="pso")
                                for sp in range(nsup):
                                    c0 = sp * PSUP
                                    c1 = min(MCHA, c0 + PSUP)
                                    wp = wp6.tile([128, PSUP, 128], DT, name="wp6",
                                                  tag="wp6")
                                    nc.sync.dma_start(
                                        wp[:, 0:c1 - c0, :],
                                        wpr_r[:, c0:c1, nt * 128:(nt + 1) * 128])
                                    for ch in range(c0, c1):
                                        nc.tensor.matmul(
                                            pso[:], wp[:, ch - c0, :], act[:, ch, :],
                                            start=(ch == 0), stop=(ch == MCHA - 1))
                                ev = evp6.tile([128, QS], F32, name="evo", tag="evo")
                                nc.scalar.copy(ev[:], pso[:])
                                nc.sync.dma_start(outT_r[:, nt, ts * QS:(ts + 1) * QS],
                                                  ev[:])

    nc.compile()
    return nc


# ============ host side ============
_NC_CACHE = {}


def _get_nc(dims, dtype):
    key = (tuple(sorted(dims.items())), dtype)
    if key not in _NC_CACHE:
        _NC_CACHE[key] = build_nc(dims, dtype)
    return _NC_CACHE[key]


def prep_weights(dims, ln1_w, ln1_b, attn_w, attn_b, ln2_w, ln2_b, fc_w, fc_b,
                 proj_w, proj_b):
    C = dims["C"]
    M4 = 4 * C
    CHA = C // 128 + 1
    MCHA = M4 // 128 + 1
    smscale = np.float32(1.0 / math.sqrt(dims["HD"]))

    def aug(wpart, bpart, g, bvec, scale=1.0):
        ncols = wpart.shape[1]
        out = np.zeros((CHA * 128, ncols), np.float32)
        wt = (g[:, None] * wpart).astype(np.float32)
        out[:C] = wt
        out[C] = wt.sum(0)
        out[C + 1] = bvec @ wpart + bpart
        return np.ascontiguousarray(out * np.float32(scale))

    wq = aug(attn_w[:, 0:C], attn_b[0:C], ln1_w, ln1_b, smscale)
    wk = aug(attn_w[:, C:2 * C], attn_b[C:2 * C], ln1_w, ln1_b)
    wv = aug(attn_w[:, 2 * C:3 * C], attn_b[2 * C:3 * C], ln1_w, ln1_b)
    wfc = aug(fc_w, fc_b, ln2_w, ln2_b)
    wpr = np.zeros((MCHA * 128, C), np.float32)
    wpr[:M4] = proj_w
    wpr[M4] = proj_b
    return wq, wk, wv, wfc, np.ascontiguousarray(wpr)


def kernel(x, ln1_w, ln1_b, attn_w, attn_b, ln2_w, ln2_b, fc_w, fc_b, proj_w,
           proj_b, dims=None, n_cores=None, trace=False, dtype="fp16"):
    dims = dims or FULL_DIMS
    n_cores = n_cores if n_cores is not None else N_CORES
    B, T, C = dims["B"], dims["T"], dims["C"]
    TQ = T // 2
    x = np.asarray(x, np.float32)
    args = [np.asarray(a, np.float32) for a in
            (ln1_w, ln1_b, attn_w, attn_b, ln2_w, ln2_b, fc_w, fc_b, proj_w, proj_b)]
    wq, wk, wv, wfc, wpr = prep_weights(dims, *args)
    if dtype == "bf16":
        import ml_dtypes
        cast = lambda a: np.ascontiguousarray(a.astype(ml_dtypes.bfloat16))
    elif dtype == "fp16":
        cast = lambda a: np.ascontiguousarray(a.astype(np.float16))
    else:
        cast = lambda a: np.ascontiguousarray(a)
    wq, wk, wv, wfc, wpr = cast(wq), cast(wk), cast(wv), cast(wfc), cast(wpr)
    nc = _get_nc(dims, dtype)

    in_maps = []
    for c in range(n_cores):
        b, p = c // 2, c % 2
        xt = np.ascontiguousarray(x[b].T)
        if p == 1:
            xt = np.ascontiguousarray(np.concatenate([xt[:, TQ:], xt[:, :TQ]], axis=1))
        in_maps.append({
            "xT": cast(xt), "wq": wq, "wk": wk, "wv": wv, "wfc": wfc, "wpr": wpr,
            "pbias": np.array([[0.0 if p == 1 else -10000.0]], np.float32),
        })

    res = run_bass_kernel_spmd(nc, in_maps, core_ids=list(range(n_cores)), trace=trace)
    out = np.empty((B, T, C), np.float32)
    for c in range(n_cores):
        b, p = c // 2, c % 2
        out[b, p * TQ:(p + 1) * TQ, :] = res.results[c]["outT"].T
    if trace:
        return out, res
    return out


# revision 11
# speedup vs baseline: 1.4602x; 1.0128x over previous
"""Trainium2 Bass kernel for nn_Block_56667798504032 (dense transformer block).

Sharding: 8 cores = 4 batches x 2 query-halves. Each core computes LN1+qkv
for its batch's 2048 tokens, causal attention + LN2 + MLP for its own 1024
query tokens. Single SPMD program; per-core behavior differs only via data
(token permutation + one exp-bias scalar).

Tricks:
- Both LayerNorms are folded into the following matmuls via an augmented
  contraction chunk (host-precomputed u/z weight rows; device-computed
  (-mu*rstd, 1) data rows). No normalized activations are materialized.
- No on-device transposes: host supplies x^T; scores are computed
  transposed (k on partitions) so softmax runs on the free axis; y is
  produced transposed (c on partitions), ready for fc; final output is
  produced transposed and un-transposed on host.
- Per-head softmax denominators: computed via a ones-column matmul and
  divided out during the attention PSUM eviction (reciprocal+broadcast
  overlap under the PE's next tile).
- Causal masks: static affine_select on diagonal k-tiles; other-half
  k-tiles via a data-driven exp bias (0 or -1e4).
- fp16 matmul operands (1 cycle/row like bf16, 8x the mantissa of bf16;
  fp32 PSUM accumulate). LayerNorm statistics in fp32.
"""
import math
import numpy as np

import concourse.bass as bass
import concourse.mybir as mybir
import concourse.tile as tile
from concourse import bacc
from concourse.bass_utils import run_bass_kernel_spmd

F32 = mybir.dt.float32
F32R = mybir.dt.float32r
BF16 = mybir.dt.bfloat16
FP16 = mybir.dt.float16
AF = mybir.ActivationFunctionType
DTYPES = {"f32r": F32R, "bf16": BF16, "fp16": FP16}

FULL_DIMS = dict(B=4, T=2048, C=2048, NH=16, HD=128)
EPS = 1e-5
N_CORES = 8


def build_nc(dims, dtype="fp16"):
    DT = DTYPES[dtype]
    B, T, C, NH, HD = dims["B"], dims["T"], dims["C"], dims["NH"], dims["HD"]
    assert HD == 128
    TQ = T // 2
    CH = C // 128
    CHA = CH + 1
    M4 = 4 * C
    MCH = M4 // 128
    MCHA = MCH + 1
    QS = 512
    NQS = TQ // QS
    NKT = T // 128
    OWN_KT = TQ // 128
    DIAG = QS // 128
    NMT = M4 // 128
    NNT = C // 128
    PSUP = 16

    nc = bacc.Bacc(None, target_bir_lowering=False)
    with tile.TileContext(nc) as tc:
        with tc.tile_pool(name="dram", bufs=1, space="DRAM") as dram:
            xT = dram.tile([C, T], DT, kind="ExternalInput", uniquify=False, name="xT")
            wq = dram.tile([CHA * 128, C], DT, kind="ExternalInput", uniquify=False, name="wq")
            wk = dram.tile([CHA * 128, C], DT, kind="ExternalInput", uniquify=False, name="wk")
            wv = dram.tile([CHA * 128, C], DT, kind="ExternalInput", uniquify=False, name="wv")
            wfc = dram.tile([CHA * 128, M4], DT, kind="ExternalInput", uniquify=False, name="wfc")
            wpr = dram.tile([MCHA * 128, C], DT, kind="ExternalInput", uniquify=False, name="wpr")
            pbias = dram.tile([1, 1], F32, kind="ExternalInput", uniquify=False, name="pbias")
            outT = dram.tile([C, TQ], F32, kind="ExternalOutput", uniquify=False, name="outT")
            kd = dram.tile([NH, 128, T], DT, name="kd")
            qd = dram.tile([NH, 128, TQ], DT, name="qd")
            vdr = dram.tile([T // 128, 128, C], DT, name="vdr")

            xT_r = xT[:].rearrange("(ch p) t -> p ch t", p=128)
            wq_r = wq[:].rearrange("(ch p) n -> p ch n", p=128)
            wk_r = wk[:].rearrange("(ch p) n -> p ch n", p=128)
            wv_r = wv[:].rearrange("(ch p) n -> p ch n", p=128)
            wfc_r = wfc[:].rearrange("(ch p) n -> p ch n", p=128)
            wpr_r = wpr[:].rearrange("(ch p) n -> p ch n", p=128)
            outT_r = outT[:].rearrange("(nt p) t -> p nt t", p=128)

            # ====== Phases 0-2: LN1 stats, scaled Xs, qkv projections ======
            with tc.tile_pool(name="sb_xs", bufs=1) as sbxs:
                xs = sbxs.tile([128, CHA, T], DT, name="xs")

                with (
                    tc.tile_pool(name="p0_sb", bufs=1) as sb0,
                    tc.tile_pool(name="p0_st", bufs=4) as st0,
                    tc.tile_pool(name="p0_ps", bufs=2, space="PSUM") as ps0,
                ):
                    ones0f = sb0.tile([128, 1], F32, name="ones0f")
                    nc.vector.memset(ones0f[:], 1.0)
                    ones0 = sb0.tile([128, 1], DT, name="ones0")
                    nc.vector.tensor_copy(ones0[:], ones0f[:])
                    eps_t = sb0.tile([1, 1], F32, name="eps_t")
                    nc.vector.memset(eps_t[:], EPS)
                    sum_row = sb0.tile([1, T], F32, name="sum_row")
                    sq_row = sb0.tile([1, T], F32, name="sq_row")
                    rstd_row = sb0.tile([1, T], F32, name="rstd_row")
                    tmp_row = sb0.tile([1, T], F32, name="tmp_row")

                    for sl in range(T // 512):
                        ps_s = ps0.tile([1, 512], F32, name="ps_s", tag="ps_s")
                        ps_q = ps0.tile([1, 512], F32, name="ps_q", tag="ps_q")
                        csl = slice(sl * 512, (sl + 1) * 512)
                        for ch in range(CH):
                            xc = st0.tile([128, 512], DT, name="xc", tag="xc")
                            nc.sync.dma_start(xc[:], xT_r[:, ch, csl])
                            nc.tensor.matmul(ps_s[:], ones0[:], xc[:],
                                             start=(ch == 0), stop=(ch == CH - 1))
                            x2 = st0.tile([128, 512], DT, name="x2", tag="x2")
                            nc.scalar.activation(x2[:], xc[:], AF.Square)
                            nc.tensor.matmul(ps_q[:], ones0[:], x2[:],
                                             start=(ch == 0), stop=(ch == CH - 1))
                        nc.scalar.copy(sum_row[:, csl], ps_s[:])
                        nc.scalar.copy(sq_row[:, csl], ps_q[:])

                    nc.scalar.mul(sum_row[:], sum_row[:], 1.0 / C)        # mean
                    nc.scalar.mul(sq_row[:], sq_row[:], 1.0 / C)          # E[x2]
                    nc.vector.tensor_mul(tmp_row[:], sum_row[:], sum_row[:])
                    nc.vector.tensor_sub(sq_row[:], sq_row[:], tmp_row[:])  # var
                    nc.scalar.activation(rstd_row[:], sq_row[:], AF.Sqrt, bias=eps_t[:])
                    nc.vector.reciprocal(rstd_row[:], rstd_row[:])        # rstd
                    nc.vector.tensor_mul(tmp_row[:], sum_row[:], rstd_row[:])
                    nc.scalar.mul(tmp_row[:], tmp_row[:], -1.0)           # -mu*rstd

                    bc = sb0.tile([128, T], DT, name="bc")
                    rstd_dt = sb0.tile([1, T], DT, name="rstd_dt")
                    nc.vector.tensor_copy(rstd_dt[:], rstd_row[:])
                    nc.gpsimd.partition_broadcast(bc[:], rstd_dt[:])
                    for ch in range(CH):
                        xc2 = st0.tile([128, T], DT, name="xc2", tag="xc2")
                        nc.sync.dma_start(xc2[:], xT_r[:, ch, :])
                        nc.vector.tensor_mul(xs[:, ch, :], xc2[:], bc[:])
                    zf0 = sb0.tile([128, T], F32, name="zf0")
                    nc.vector.memset(zf0[:], 0.0)
                    nc.vector.memset(zf0[0:2, :], 1.0)
                    nc.vector.tensor_copy(xs[:, CH, :], zf0[:])
                    nc.vector.tensor_copy(xs[0:1, CH, :], tmp_row[:])

                with (
                    tc.tile_pool(name="p2_w", bufs=3) as wp2,
                    tc.tile_pool(name="p2_wv", bufs=2) as wvp2,
                    tc.tile_pool(name="p2_ev", bufs=4) as evp2,
                    tc.tile_pool(name="p2_ps", bufs=1, space="PSUM") as psk,
                    tc.tile_pool(name="p2_psv", bufs=3, space="PSUM") as psv2,
                ):
                    for (wr, nsl, dst) in ((wk_r, T // 512, kd), (wq_r, TQ // 512, qd)):
                        for ot in range(NNT):
                            wt = wp2.tile([128, CHA, 128], DT, name="wt", tag="wblk")
                            nc.sync.dma_start(wt[:], wr[:, :, ot * 128:(ot + 1) * 128])
                            # slice-inner so one LDWEIGHTS serves nsl matmuls
                            pss_l = [psk.tile([128, 512], F32, name=f"pk{i}", tag=f"pk{i}")
                                     for i in range(nsl)]
                            for ch in range(CHA):
                                for sl in range(nsl):
                                    nc.tensor.matmul(
                                        pss_l[sl][:], wt[:, ch, :],
                                        xs[:, ch, sl * 512:(sl + 1) * 512],
                                        start=(ch == 0), stop=(ch == CHA - 1))
                            for sl in range(nsl):
                                ev = evp2.tile([128, 512], DT, name="ev", tag="ev")
                                nc.scalar.copy(ev[:], pss_l[sl][:])
                                nc.sync.dma_start(dst[ot, :, sl * 512:(sl + 1) * 512], ev[:])
                    for osl in range(C // 512):
                        wvt = wvp2.tile([128, CHA, 512], DT, name="wvt", tag="wv")
                        nc.sync.dma_start(wvt[:], wv_r[:, :, osl * 512:(osl + 1) * 512])
                        for tt in range(T // 128):
                            psv = psv2.tile([128, 512], F32, name="psv", tag="psv")
                            for ch in range(CHA):
                                nc.tensor.matmul(
                                    psv[:], xs[:, ch, tt * 128:(tt + 1) * 128],
                                    wvt[:, ch, :],
                                    start=(ch == 0), stop=(ch == CHA - 1))
                            ev = evp2.tile([128, 512], DT, name="evv", tag="ev")
                            nc.scalar.copy(ev[:], psv[:])
                            nc.sync.dma_start(vdr[tt, :, osl * 512:(osl + 1) * 512], ev[:])

            # ============ Phase 3: causal attention (normalized inline) ====
            with tc.tile_pool(name="p3_sb", bufs=1) as sb3:
                yT = sb3.tile([128, NH, TQ], DT, name="yT")
                ysr = sb3.tile([128, CHA, TQ], DT, name="ysr")
                ones1f = sb3.tile([128, 1], F32, name="ones1f")
                nc.vector.memset(ones1f[:], 1.0)
                ones1 = sb3.tile([128, 1], DT, name="ones1")
                nc.vector.tensor_copy(ones1[:], ones1f[:])
                pbr = sb3.tile([1, 1], F32, name="pbr")
                nc.sync.dma_start(pbr[:], pbias[:])
                pb = sb3.tile([128, 1], F32, name="pb")
                nc.gpsimd.partition_broadcast(pb[:], pbr[:])

                with (
                    tc.tile_pool(name="p3_h", bufs=2) as hp3,
                    tc.tile_pool(name="p3_e", bufs=4) as ep3,
                    tc.tile_pool(name="p3_r", bufs=3) as rp3,
                    tc.tile_pool(name="p3_psy", bufs=2, space="PSUM") as psy3,
                    tc.tile_pool(name="p3_psd", bufs=2, space="PSUM") as psd3,
                    tc.tile_pool(name="p3_pss", bufs=3, space="PSUM") as pss3,
                ):
                    for h in range(NH):
                        kh = hp3.tile([128, T], DT, name="kh", tag="kh")
                        nc.sync.dma_start(kh[:], kd[h, :, :])
                        qh = hp3.tile([128, TQ], DT, name="qh", tag="qh")
                        nc.sync.dma_start(qh[:], qd[h, :, :])
                        vh = hp3.tile([128, T // 128, 128], DT, name="vh", tag="vh")
                        nc.sync.dma_start(
                            vh[:],
                            vdr[:].rearrange("t p n -> p t n")[:, :, h * 128:(h + 1) * 128])

                        for s in range(NQS):
                            ktiles = (list(range(0, DIAG * s + DIAG))
                                      + list(range(OWN_KT, NKT)))
                            psy = psy3.tile([128, QS], F32, name="psy", tag="psy")
                            psd = psd3.tile([1, QS], F32, name="psd", tag="psd")
                            for idx, j in enumerate(ktiles):
                                pss = pss3.tile([128, QS], F32, name="pss", tag="pss")
                                nc.tensor.matmul(pss[:], kh[:, j * 128:(j + 1) * 128],
                                                 qh[:, s * QS:(s + 1) * QS],
                                                 start=True, stop=True)
                                es = ep3.tile([128, QS], DT, name="es", tag="es")
                                if j >= OWN_KT:
                                    nc.scalar.activation(es[:], pss[:], AF.Exp, bias=pb[:])
                                else:
                                    nc.scalar.activation(es[:], pss[:], AF.Exp)
                                if DIAG * s <= j < DIAG * s + DIAG:
                                    d = j - DIAG * s
                                    nc.gpsimd.affine_select(
                                        es[:], es[:], [[1, QS]], mybir.AluOpType.is_ge,
                                        0.0, base=-128 * d, channel_multiplier=-1)
                                first, last = (idx == 0), (idx == len(ktiles) - 1)
                                nc.tensor.matmul(psy[:], vh[:, j, :], es[:],
                                                 start=first, stop=last)
                                nc.tensor.matmul(psd[:], ones1[:], es[:],
                                                 start=first, stop=last)
                            # normalize by softmax denominator during eviction
                            rdr = rp3.tile([1, QS], F32, name="rdr", tag="rdr")
                            nc.vector.reciprocal(rdr[:], psd[:])
                            bcd = rp3.tile([128, QS], F32, name="bcd", tag="bcd")
                            nc.gpsimd.partition_broadcast(bcd[:], rdr[:])
                            nc.vector.tensor_mul(yT[:, h, s * QS:(s + 1) * QS],
                                                 psy[:], bcd[:])

                # ============ Phase 4/5: LN2 stats + y scaling ============
                with (
                    tc.tile_pool(name="p45_sb", bufs=1) as sb45,
                    tc.tile_pool(name="p45_w", bufs=2) as w45,
                    tc.tile_pool(name="p45_ps", bufs=2, space="PSUM") as ps45,
                ):
                    m_row = sb45.tile([1, TQ], F32, name="m_row")
                    s_row = sb45.tile([1, TQ], F32, name="s_row")
                    t_row = sb45.tile([1, TQ], F32, name="t_row")
                    r2_row = sb45.tile([1, TQ], F32, name="r2_row")
                    nm2_row = sb45.tile([1, TQ], F32, name="nm2_row")
                    eps2_t = sb45.tile([1, 1], F32, name="eps2_t")
                    nc.vector.memset(eps2_t[:], EPS)
                    for s in range(NQS):
                        qsl = slice(s * QS, (s + 1) * QS)
                        ps_m = ps45.tile([1, QS], F32, name="ps_m", tag="ps_m")
                        ps_q2 = ps45.tile([1, QS], F32, name="ps_q2", tag="ps_q2")
                        for h in range(NH):
                            y2 = w45.tile([128, QS], DT, name="y2", tag="y2")
                            nc.scalar.activation(y2[:], yT[:, h, qsl], AF.Square)
                            nc.tensor.matmul(ps_m[:], ones1[:], yT[:, h, qsl],
                                             start=(h == 0), stop=(h == NH - 1))
                            nc.tensor.matmul(ps_q2[:], ones1[:], y2[:],
                                             start=(h == 0), stop=(h == NH - 1))
                        nc.scalar.copy(m_row[:, qsl], ps_m[:])
                        nc.scalar.copy(s_row[:, qsl], ps_q2[:])
                    nc.scalar.mul(m_row[:], m_row[:], 1.0 / C)
                    nc.scalar.mul(s_row[:], s_row[:], 1.0 / C)
                    nc.vector.tensor_mul(t_row[:], m_row[:], m_row[:])
                    nc.vector.tensor_sub(s_row[:], s_row[:], t_row[:])
                    nc.scalar.activation(r2_row[:], s_row[:], AF.Sqrt, bias=eps2_t[:])
                    nc.vector.reciprocal(r2_row[:], r2_row[:])
                    nc.vector.tensor_mul(nm2_row[:], m_row[:], r2_row[:])
                    nc.scalar.mul(nm2_row[:], nm2_row[:], -1.0)

                    r2d = sb45.tile([1, TQ], DT, name="r2d")
                    nc.vector.tensor_copy(r2d[:], r2_row[:])
                    bc2 = sb45.tile([128, TQ], DT, name="bc2")
                    nc.gpsimd.partition_broadcast(bc2[:], r2d[:])
                    for h in range(NH):
                        nc.vector.tensor_mul(ysr[:, h, :], yT[:, h, :], bc2[:])
                    zf45 = sb45.tile([128, TQ], F32, name="zf45")
                    nc.vector.memset(zf45[:], 0.0)
                    nc.vector.memset(zf45[0:2, :], 1.0)
                    nc.vector.tensor_copy(ysr[:, CH, :], zf45[:])
                    nc.vector.tensor_copy(ysr[0:1, CH, :], nm2_row[:])

                # ============ Phase 6: MLP ============
                for ts in range(NQS):
                    with tc.tile_pool(name=f"p6_act{ts}", bufs=1) as sb6:
                        act = sb6.tile([128, MCHA, QS], DT, name="act")
                        zf6 = sb6.tile([128, QS], F32, name="zf6")
                        nc.vector.memset(zf6[:], 0.0)
                        nc.vector.memset(zf6[0:1, :], 1.0)
                        nc.vector.tensor_copy(act[:, MCH, :], zf6[:])
                        with (
                            tc.tile_pool(name=f"p6f_w{ts}", bufs=3) as wf6,
                            tc.tile_pool(name=f"p6f_ps{ts}", bufs=3, space="PSUM") as psf6,
                        ):
                            for mt in range(NMT):
                                wt = wf6.tile([128, CHA, 128], DT, name="wt6", tag="w6")
                                nc.sync.dma_start(wt[:], wfc_r[:, :, mt * 128:(mt + 1) * 128])
                                psf = psf6.tile([128, QS], F32, name="psf", tag="psf")
                                for ch in range(CHA):
                                    nc.tensor.matmul(
                                        psf[:], wt[:, ch, :],
                                        ysr[:, ch, ts * QS:(ts + 1) * QS],
                                        start=(ch == 0), stop=(ch == CHA - 1))
                                nc.scalar.activation(act[:, mt, :], psf[:], AF.Gelu)
                        with (
                            tc.tile_pool(name=f"p6p_w{ts}", bufs=3) as wp6,
                            tc.tile_pool(name=f"p6p_ev{ts}", bufs=3) as evp6,
                            tc.tile_pool(name=f"p6p_ps{ts}", bufs=3, space="PSUM") as psp6,
                        ):
                            nsup = -(-MCHA // PSUP)
                            for nt in range(NNT):
                                pso = psp6.tile([128, QS], F32, name="pso", tag="pso")
                                for sp in range(nsup):
                                    c0 = sp * PSUP
                                    c1 = min(MCHA, c0 + PSUP)
                                    wp = wp6.tile([128, PSUP, 128], DT, name="wp6",
                                                  tag="wp6")
                                    nc.sync.dma_start(
                                        wp[:, 0:c1 - c0, :],
                                        wpr_r[:, c0:c1, nt * 128:(nt + 1) * 128])
                                    for ch in range(c0, c1):
                                        nc.tensor.matmul(
                                            pso[:], wp[:, ch - c0, :], act[:, ch, :],
                                            start=(ch == 0), stop=(ch == MCHA - 1))
                                ev = evp6.tile([128, QS], F32, name="evo", tag="evo")
                                nc.scalar.copy(ev[:], pso[:])
                                nc.sync.dma_start(outT_r[:, nt, ts * QS:(ts + 1) * QS],
                                                  ev[:])

    nc.compile()
    return nc


# ============ host side ============
_NC_CACHE = {}


def _get_nc(dims, dtype):
    key = (tuple(sorted(dims.items())), dtype)
    if key not in _NC_CACHE:
        _NC_CACHE[key] = build_nc(dims, dtype)
    return _NC_CACHE[key]


def prep_weights(dims, ln1_w, ln1_b, attn_w, attn_b, ln2_w, ln2_b, fc_w, fc_b,
                 proj_w, proj_b):
    C = dims["C"]
    M4 = 4 * C
    CHA = C // 128 + 1
    MCHA = M4 // 128 + 1
    smscale = np.float32(1.0 / math.sqrt(dims["HD"]))

    def aug(wpart, bpart, g, bvec, scale=1.0):
        ncols = wpart.shape[1]
        out = np.zeros((CHA * 128, ncols), np.float32)
        wt = (g[:, None] * wpart).astype(np.float32)
        out[:C] = wt
        out[C] = wt.sum(0)
        out[C + 1] = bvec @ wpart + bpart
        return np.ascontiguousarray(out * np.float32(scale))

    wq = aug(attn_w[:, 0:C], attn_b[0:C], ln1_w, ln1_b, smscale)
    wk = aug(attn_w[:, C:2 * C], attn_b[C:2 * C], ln1_w, ln1_b)
    wv = aug(attn_w[:, 2 * C:3 * C], attn_b[2 * C:3 * C], ln1_w, ln1_b)
    wfc = aug(fc_w, fc_b, ln2_w, ln2_b)
    wpr = np.zeros((MCHA * 128, C), np.float32)
    wpr[:M4] = proj_w
    wpr[M4] = proj_b
    return wq, wk, wv, wfc, np.ascontiguousarray(wpr)


def kernel(x, ln1_w, ln1_b, attn_w, attn_b, ln2_w, ln2_b, fc_w, fc_b, proj_w,
           proj_b, dims=None, n_cores=None, trace=False, dtype="fp16"):
    dims = dims or FULL_DIMS
    n_cores = n_cores if n_cores is not None else N_CORES
    B, T, C = dims["B"], dims["T"], dims["C"]
    TQ = T // 2
    x = np.asarray(x, np.float32)
    args = [np.asarray(a, np.float32) for a in
            (ln1_w, ln1_b, attn_w, attn_b, ln2_w, ln2_b, fc_w, fc_b, proj_w, proj_b)]
    wq, wk, wv, wfc, wpr = prep_weights(dims, *args)
    if dtype == "bf16":
        import ml_dtypes
        cast = lambda a: np.ascontiguousarray(a.astype(ml_dtypes.bfloat16))
    elif dtype == "fp16":
        cast = lambda a: np.ascontiguousarray(a.astype(np.float16))
    else:
        cast = lambda a: np.ascontiguousarray(a)
    wq, wk, wv, wfc, wpr = cast(wq), cast(wk), cast(wv), cast(wfc), cast(wpr)
    nc = _get_nc(dims, dtype)

    in_maps = []
    for c in range(n_cores):
        b, p = c // 2, c % 2
        xt = np.ascontiguousarray(x[b].T)
        if p == 1:
            xt = np.ascontiguousarray(np.concatenate([xt[:, TQ:], xt[:, :TQ]], axis=1))
        in_maps.append({
            "xT": cast(xt), "wq": wq, "wk": wk, "wv": wv, "wfc": wfc, "wpr": wpr,
            "pbias": np.array([[0.0 if p == 1 else -10000.0]], np.float32),
        })

    res = run_bass_kernel_spmd(nc, in_maps, core_ids=list(range(n_cores)), trace=trace)
    out = np.empty((B, T, C), np.float32)
    for c in range(n_cores):
        b, p = c // 2, c % 2
        out[b, p * TQ:(p + 1) * TQ, :] = res.results[c]["outT"].T
    if trace:
        return out, res
    return out


# revision 12
# speedup vs baseline: 1.5311x; 1.0486x over previous
"""Trainium2 Bass kernel for nn_Block_56667798504032 (dense transformer block).

Sharding: 8 cores = 4 batches x 2 query-halves. Each core computes LN1+qkv
for its batch's 2048 tokens, causal attention + LN2 + MLP for its own 1024
query tokens. Single SPMD program; per-core behavior differs only via data
(token permutation + one exp-bias scalar).

Tricks:
- Both LayerNorms are folded into the following matmuls via an augmented
  contraction chunk (host-precomputed u/z weight rows; device-computed
  (-mu*rstd, 1) data rows). No normalized activations are materialized.
- No on-device transposes: host supplies x^T; scores are computed
  transposed (k on partitions) so softmax runs on the free axis; y is
  produced transposed (c on partitions), ready for fc; final output is
  produced transposed and un-transposed on host.
- Per-head softmax denominators: computed via a ones-column matmul and
  divided out during the attention PSUM eviction (reciprocal+broadcast
  overlap under the PE's next tile).
- Causal masks: static affine_select on diagonal k-tiles; other-half
  k-tiles via a data-driven exp bias (0 or -1e4).
- fp16 matmul operands (1 cycle/row like bf16, 8x the mantissa of bf16;
  fp32 PSUM accumulate). LayerNorm statistics in fp32.
"""
import math
import numpy as np

import concourse.bass as bass
import concourse.mybir as mybir
import concourse.tile as tile
from concourse import bacc
from concourse.bass_utils import run_bass_kernel_spmd

F32 = mybir.dt.float32
F32R = mybir.dt.float32r
BF16 = mybir.dt.bfloat16
FP16 = mybir.dt.float16
AF = mybir.ActivationFunctionType
DTYPES = {"f32r": F32R, "bf16": BF16, "fp16": FP16}

FULL_DIMS = dict(B=4, T=2048, C=2048, NH=16, HD=128)
EPS = 1e-5
N_CORES = 8


def build_nc(dims, dtype="fp16"):
    DT = DTYPES[dtype]
    B, T, C, NH, HD = dims["B"], dims["T"], dims["C"], dims["NH"], dims["HD"]
    assert HD == 128
    TQ = T // 2
    CH = C // 128
    CHA = CH + 1
    M4 = 4 * C
    MCH = M4 // 128
    MCHA = MCH + 1
    QS = 512
    NQS = TQ // QS
    NKT = T // 128
    OWN_KT = TQ // 128
    DIAG = QS // 128
    NMT = M4 // 128
    NNT = C // 128
    PSUP = 16

    nc = bacc.Bacc(None, target_bir_lowering=False)
    with tile.TileContext(nc) as tc:
        with tc.tile_pool(name="dram", bufs=1, space="DRAM") as dram:
            xT = dram.tile([C, T], DT, kind="ExternalInput", uniquify=False, name="xT")
            wq = dram.tile([CHA * 128, C], DT, kind="ExternalInput", uniquify=False, name="wq")
            wk = dram.tile([CHA * 128, C], DT, kind="ExternalInput", uniquify=False, name="wk")
            wv = dram.tile([CHA * 128, C], DT, kind="ExternalInput", uniquify=False, name="wv")
            wfc = dram.tile([CHA * 128, M4], DT, kind="ExternalInput", uniquify=False, name="wfc")
            wpr = dram.tile([MCHA * 128, C], DT, kind="ExternalInput", uniquify=False, name="wpr")
            pbias = dram.tile([1, 1], F32, kind="ExternalInput", uniquify=False, name="pbias")
            outT = dram.tile([C, TQ], F32, kind="ExternalOutput", uniquify=False, name="outT")
            kd = dram.tile([NH, 128, T], DT, name="kd")
            qd = dram.tile([NH, 128, TQ], DT, name="qd")
            vdr = dram.tile([T // 128, 128, C], DT, name="vdr")

            xT_r = xT[:].rearrange("(ch p) t -> p ch t", p=128)
            wq_r = wq[:].rearrange("(ch p) n -> p ch n", p=128)
            wk_r = wk[:].rearrange("(ch p) n -> p ch n", p=128)
            wv_r = wv[:].rearrange("(ch p) n -> p ch n", p=128)
            wfc_r = wfc[:].rearrange("(ch p) n -> p ch n", p=128)
            wpr_r = wpr[:].rearrange("(ch p) n -> p ch n", p=128)
            outT_r = outT[:].rearrange("(nt p) t -> p nt t", p=128)

            # ====== Phases 0-2: LN1 stats, scaled Xs, qkv projections ======
            with tc.tile_pool(name="sb_xs", bufs=1) as sbxs:
                xs = sbxs.tile([128, CHA, T], DT, name="xs")

                with (
                    tc.tile_pool(name="p0_sb", bufs=1) as sb0,
                    tc.tile_pool(name="p0_st", bufs=8) as st0,
                    tc.tile_pool(name="p0_ps", bufs=2, space="PSUM") as ps0,
                ):
                    ones0f = sb0.tile([128, 1], F32, name="ones0f")
                    nc.vector.memset(ones0f[:], 1.0)
                    ones0 = sb0.tile([128, 1], DT, name="ones0")
                    nc.vector.tensor_copy(ones0[:], ones0f[:])
                    eps_t = sb0.tile([1, 1], F32, name="eps_t")
                    nc.vector.memset(eps_t[:], EPS)
                    sum_row = sb0.tile([1, T], F32, name="sum_row")
                    sq_row = sb0.tile([1, T], F32, name="sq_row")
                    rstd_row = sb0.tile([1, T], F32, name="rstd_row")
                    tmp_row = sb0.tile([1, T], F32, name="tmp_row")

                    for sl in range(T // 512):
                        ps_s = ps0.tile([1, 512], F32, name="ps_s", tag="ps_s")
                        ps_q = ps0.tile([1, 512], F32, name="ps_q", tag="ps_q")
                        csl = slice(sl * 512, (sl + 1) * 512)
                        for ch in range(CH):
                            xc = st0.tile([128, 512], DT, name="xc", tag="xc")
                            nc.sync.dma_start(xc[:], xT_r[:, ch, csl])
                            nc.tensor.matmul(ps_s[:], ones0[:], xc[:],
                                             start=(ch == 0), stop=(ch == CH - 1))
                            x2 = st0.tile([128, 512], DT, name="x2", tag="x2")
                            nc.scalar.activation(x2[:], xc[:], AF.Square)
                            nc.tensor.matmul(ps_q[:], ones0[:], x2[:],
                                             start=(ch == 0), stop=(ch == CH - 1))
                        nc.scalar.copy(sum_row[:, csl], ps_s[:])
                        nc.scalar.copy(sq_row[:, csl], ps_q[:])

                    nc.scalar.mul(sum_row[:], sum_row[:], 1.0 / C)        # mean
                    nc.scalar.mul(sq_row[:], sq_row[:], 1.0 / C)          # E[x2]
                    nc.vector.tensor_mul(tmp_row[:], sum_row[:], sum_row[:])
                    nc.vector.tensor_sub(sq_row[:], sq_row[:], tmp_row[:])  # var
                    nc.scalar.activation(rstd_row[:], sq_row[:], AF.Sqrt, bias=eps_t[:])
                    nc.vector.reciprocal(rstd_row[:], rstd_row[:])        # rstd
                    nc.vector.tensor_mul(tmp_row[:], sum_row[:], rstd_row[:])
                    nc.scalar.mul(tmp_row[:], tmp_row[:], -1.0)           # -mu*rstd

                    bc = sb0.tile([128, T], DT, name="bc")
                    rstd_dt = sb0.tile([1, T], DT, name="rstd_dt")
                    nc.vector.tensor_copy(rstd_dt[:], rstd_row[:])
                    nc.gpsimd.partition_broadcast(bc[:], rstd_dt[:])
                    for ch in range(CH):
                        xc2 = st0.tile([128, T], DT, name="xc2", tag="xc2")
                        nc.sync.dma_start(xc2[:], xT_r[:, ch, :])
                        nc.vector.tensor_mul(xs[:, ch, :], xc2[:], bc[:])
                    zf0 = sb0.tile([128, T], F32, name="zf0")
                    nc.vector.memset(zf0[:], 0.0)
                    nc.vector.memset(zf0[0:2, :], 1.0)
                    nc.vector.tensor_copy(xs[:, CH, :], zf0[:])
                    nc.vector.tensor_copy(xs[0:1, CH, :], tmp_row[:])

                with (
                    tc.tile_pool(name="p2_w", bufs=3) as wp2,
                    tc.tile_pool(name="p2_wv", bufs=2) as wvp2,
                    tc.tile_pool(name="p2_ev", bufs=4) as evp2,
                    tc.tile_pool(name="p2_ps", bufs=1, space="PSUM") as psk,
                    tc.tile_pool(name="p2_psv", bufs=3, space="PSUM") as psv2,
                ):
                    for (wr, nsl, dst) in ((wk_r, T // 512, kd), (wq_r, TQ // 512, qd)):
                        for ot in range(NNT):
                            wt = wp2.tile([128, CHA, 128], DT, name="wt", tag="wblk")
                            nc.sync.dma_start(wt[:], wr[:, :, ot * 128:(ot + 1) * 128])
                            # slice-inner so one LDWEIGHTS serves nsl matmuls
                            pss_l = [psk.tile([128, 512], F32, name=f"pk{i}", tag=f"pk{i}")
                                     for i in range(nsl)]
                            for ch in range(CHA):
                                for sl in range(nsl):
                                    nc.tensor.matmul(
                                        pss_l[sl][:], wt[:, ch, :],
                                        xs[:, ch, sl * 512:(sl + 1) * 512],
                                        start=(ch == 0), stop=(ch == CHA - 1))
                            for sl in range(nsl):
                                ev = evp2.tile([128, 512], DT, name="ev", tag="ev")
                                nc.scalar.copy(ev[:], pss_l[sl][:])
                                nc.sync.dma_start(dst[ot, :, sl * 512:(sl + 1) * 512], ev[:])
                    for osl in range(C // 512):
                        wvt = wvp2.tile([128, CHA, 512], DT, name="wvt", tag="wv")
                        nc.sync.dma_start(wvt[:], wv_r[:, :, osl * 512:(osl + 1) * 512])
                        for tt in range(T // 128):
                            psv = psv2.tile([128, 512], F32, name="psv", tag="psv")
                            for ch in range(CHA):
                                nc.tensor.matmul(
                                    psv[:], xs[:, ch, tt * 128:(tt + 1) * 128],
                                    wvt[:, ch, :],
                                    start=(ch == 0), stop=(ch == CHA - 1))
                            ev = evp2.tile([128, 512], DT, name="evv", tag="ev")
                            nc.scalar.copy(ev[:], psv[:])
                            nc.sync.dma_start(vdr[tt, :, osl * 512:(osl + 1) * 512], ev[:])

            # ============ Phase 3: causal attention (normalized inline) ====
            with tc.tile_pool(name="p3_sb", bufs=1) as sb3:
                yT = sb3.tile([128, NH, TQ], DT, name="yT")
                ysr = sb3.tile([128, CHA, TQ], DT, name="ysr")
                ones1f = sb3.tile([128, 1], F32, name="ones1f")
                nc.vector.memset(ones1f[:], 1.0)
                ones1 = sb3.tile([128, 1], DT, name="ones1")
                nc.vector.tensor_copy(ones1[:], ones1f[:])
                pbr = sb3.tile([1, 1], F32, name="pbr")
                nc.sync.dma_start(pbr[:], pbias[:])
                pb = sb3.tile([128, 1], F32, name="pb")
                nc.gpsimd.partition_broadcast(pb[:], pbr[:])

                with (
                    tc.tile_pool(name="p3_h", bufs=2) as hp3,
                    tc.tile_pool(name="p3_e", bufs=6) as ep3,
                    tc.tile_pool(name="p3_r", bufs=3) as rp3,
                    tc.tile_pool(name="p3_psy", bufs=2, space="PSUM") as psy3,
                    tc.tile_pool(name="p3_psd", bufs=2, space="PSUM") as psd3,
                    tc.tile_pool(name="p3_pss", bufs=4, space="PSUM") as pss3,
                ):
                    for h in range(NH):
                        kh = hp3.tile([128, T], DT, name="kh", tag="kh")
                        nc.sync.dma_start(kh[:], kd[h, :, :])
                        qh = hp3.tile([128, TQ], DT, name="qh", tag="qh")
                        nc.sync.dma_start(qh[:], qd[h, :, :])
                        vh = hp3.tile([128, T // 128, 128], DT, name="vh", tag="vh")
                        nc.sync.dma_start(
                            vh[:],
                            vdr[:].rearrange("t p n -> p t n")[:, :, h * 128:(h + 1) * 128])

                        for s in range(NQS):
                            ktiles = (list(range(0, DIAG * s + DIAG))
                                      + list(range(OWN_KT, NKT)))
                            psy = psy3.tile([128, QS], F32, name="psy", tag="psy")
                            psd = psd3.tile([1, QS], F32, name="psd", tag="psd")
                            for idx, j in enumerate(ktiles):
                                pss = pss3.tile([128, QS], F32, name="pss", tag="pss")
                                nc.tensor.matmul(pss[:], kh[:, j * 128:(j + 1) * 128],
                                                 qh[:, s * QS:(s + 1) * QS],
                                                 start=True, stop=True)
                                es = ep3.tile([128, QS], DT, name="es", tag="es")
                                if j >= OWN_KT:
                                    nc.scalar.activation(es[:], pss[:], AF.Exp, bias=pb[:])
                                else:
                                    nc.scalar.activation(es[:], pss[:], AF.Exp)
                                if DIAG * s <= j < DIAG * s + DIAG:
                                    d = j - DIAG * s
                                    nc.gpsimd.affine_select(
                                        es[:], es[:], [[1, QS]], mybir.AluOpType.is_ge,
                                        0.0, base=-128 * d, channel_multiplier=-1)
                                first, last = (idx == 0), (idx == len(ktiles) - 1)
                                nc.tensor.matmul(psy[:], vh[:, j, :], es[:],
                                                 start=first, stop=last)
                                nc.tensor.matmul(psd[:], ones1[:], es[:],
                                                 start=first, stop=last)
                            # normalize by softmax denominator during eviction
                            rdr = rp3.tile([1, QS], F32, name="rdr", tag="rdr")
                            nc.vector.reciprocal(rdr[:], psd[:])
                            bcd = rp3.tile([128, QS], F32, name="bcd", tag="bcd")
                            nc.gpsimd.partition_broadcast(bcd[:], rdr[:])
                            nc.vector.tensor_mul(yT[:, h, s * QS:(s + 1) * QS],
                                                 psy[:], bcd[:])

                # ============ Phase 4/5: LN2 stats + y scaling ============
                with (
                    tc.tile_pool(name="p45_sb", bufs=1) as sb45,
                    tc.tile_pool(name="p45_w", bufs=2) as w45,
                    tc.tile_pool(name="p45_ps", bufs=2, space="PSUM") as ps45,
                ):
                    m_row = sb45.tile([1, TQ], F32, name="m_row")
                    s_row = sb45.tile([1, TQ], F32, name="s_row")
                    t_row = sb45.tile([1, TQ], F32, name="t_row")
                    r2_row = sb45.tile([1, TQ], F32, name="r2_row")
                    nm2_row = sb45.tile([1, TQ], F32, name="nm2_row")
                    eps2_t = sb45.tile([1, 1], F32, name="eps2_t")
                    nc.vector.memset(eps2_t[:], EPS)
                    for s in range(NQS):
                        qsl = slice(s * QS, (s + 1) * QS)
                        ps_m = ps45.tile([1, QS], F32, name="ps_m", tag="ps_m")
                        ps_q2 = ps45.tile([1, QS], F32, name="ps_q2", tag="ps_q2")
                        for h in range(NH):
                            y2 = w45.tile([128, QS], DT, name="y2", tag="y2")
                            nc.scalar.activation(y2[:], yT[:, h, qsl], AF.Square)
                            nc.tensor.matmul(ps_m[:], ones1[:], yT[:, h, qsl],
                                             start=(h == 0), stop=(h == NH - 1))
                            nc.tensor.matmul(ps_q2[:], ones1[:], y2[:],
                                             start=(h == 0), stop=(h == NH - 1))
                        nc.scalar.copy(m_row[:, qsl], ps_m[:])
                        nc.scalar.copy(s_row[:, qsl], ps_q2[:])
                    nc.scalar.mul(m_row[:], m_row[:], 1.0 / C)
                    nc.scalar.mul(s_row[:], s_row[:], 1.0 / C)
                    nc.vector.tensor_mul(t_row[:], m_row[:], m_row[:])
                    nc.vector.tensor_sub(s_row[:], s_row[:], t_row[:])
                    nc.scalar.activation(r2_row[:], s_row[:], AF.Sqrt, bias=eps2_t[:])
                    nc.vector.reciprocal(r2_row[:], r2_row[:])
                    nc.vector.tensor_mul(nm2_row[:], m_row[:], r2_row[:])
                    nc.scalar.mul(nm2_row[:], nm2_row[:], -1.0)

                    r2d = sb45.tile([1, TQ], DT, name="r2d")
                    nc.vector.tensor_copy(r2d[:], r2_row[:])
                    bc2 = sb45.tile([128, TQ], DT, name="bc2")
                    nc.gpsimd.partition_broadcast(bc2[:], r2d[:])
                    for h in range(NH):
                        nc.vector.tensor_mul(ysr[:, h, :], yT[:, h, :], bc2[:])
                    zf45 = sb45.tile([128, TQ], F32, name="zf45")
                    nc.vector.memset(zf45[:], 0.0)
                    nc.vector.memset(zf45[0:2, :], 1.0)
                    nc.vector.tensor_copy(ysr[:, CH, :], zf45[:])
                    nc.vector.tensor_copy(ysr[0:1, CH, :], nm2_row[:])

                # ============ Phase 6: MLP ============
                for ts in range(NQS):
                    with tc.tile_pool(name=f"p6_act{ts}", bufs=1) as sb6:
                        act = sb6.tile([128, MCHA, QS], DT, name="act")
                        zf6 = sb6.tile([128, QS], F32, name="zf6")
                        nc.vector.memset(zf6[:], 0.0)
                        nc.vector.memset(zf6[0:1, :], 1.0)
                        nc.vector.tensor_copy(act[:, MCH, :], zf6[:])
                        with (
                            tc.tile_pool(name=f"p6f_w{ts}", bufs=4) as wf6,
                            tc.tile_pool(name=f"p6f_ps{ts}", bufs=3, space="PSUM") as psf6,
                        ):
                            for mt in range(NMT):
                                wt = wf6.tile([128, CHA, 128], DT, name="wt6", tag="w6")
                                nc.sync.dma_start(wt[:], wfc_r[:, :, mt * 128:(mt + 1) * 128])
                                psf = psf6.tile([128, QS], F32, name="psf", tag="psf")
                                for ch in range(CHA):
                                    nc.tensor.matmul(
                                        psf[:], wt[:, ch, :],
                                        ysr[:, ch, ts * QS:(ts + 1) * QS],
                                        start=(ch == 0), stop=(ch == CHA - 1))
                                nc.scalar.activation(act[:, mt, :], psf[:], AF.Gelu)
                        with (
                            tc.tile_pool(name=f"p6p_w{ts}", bufs=4) as wp6,
                            tc.tile_pool(name=f"p6p_ev{ts}", bufs=3) as evp6,
                            tc.tile_pool(name=f"p6p_ps{ts}", bufs=3, space="PSUM") as psp6,
                        ):
                            nsup = -(-MCHA // PSUP)
                            for nt in range(NNT):
                                pso = psp6.tile([128, QS], F32, name="pso", tag="pso")
                                for sp in range(nsup):
                                    c0 = sp * PSUP
                                    c1 = min(MCHA, c0 + PSUP)
                                    wp = wp6.tile([128, PSUP, 128], DT, name="wp6",
                                                  tag="wp6")
                                    nc.sync.dma_start(
                                        wp[:, 0:c1 - c0, :],
                                        wpr_r[:, c0:c1, nt * 128:(nt + 1) * 128])
                                    for ch in range(c0, c1):
                                        nc.tensor.matmul(
                                            pso[:], wp[:, ch - c0, :], act[:, ch, :],
                                            start=(ch == 0), stop=(ch == MCHA - 1))
                                ev = evp6.tile([128, QS], F32, name="evo", tag="evo")
                                nc.scalar.copy(ev[:], pso[:])
                                nc.sync.dma_start(outT_r[:, nt, ts * QS:(ts + 1) * QS],
                                                  ev[:])

    nc.compile()
    return nc


# ============ host side ============
_NC_CACHE = {}


def _get_nc(dims, dtype):
    key = (tuple(sorted(dims.items())), dtype)
    if key not in _NC_CACHE:
        _NC_CACHE[key] = build_nc(dims, dtype)
    return _NC_CACHE[key]


def prep_weights(dims, ln1_w, ln1_b, attn_w, attn_b, ln2_w, ln2_b, fc_w, fc_b,
                 proj_w, proj_b):
    C = dims["C"]
    M4 = 4 * C
    CHA = C // 128 + 1
    MCHA = M4 // 128 + 1
    smscale = np.float32(1.0 / math.sqrt(dims["HD"]))

    def aug(wpart, bpart, g, bvec, scale=1.0):
        ncols = wpart.shape[1]
        out = np.zeros((CHA * 128, ncols), np.float32)
        wt = (g[:, None] * wpart).astype(np.float32)
        out[:C] = wt
        out[C] = wt.sum(0)
        out[C + 1] = bvec @ wpart + bpart
        return np.ascontiguousarray(out * np.float32(scale))

    wq = aug(attn_w[:, 0:C], attn_b[0:C], ln1_w, ln1_b, smscale)
    wk = aug(attn_w[:, C:2 * C], attn_b[C:2 * C], ln1_w, ln1_b)
    wv = aug(attn_w[:, 2 * C:3 * C], attn_b[2 * C:3 * C], ln1_w, ln1_b)
    wfc = aug(fc_w, fc_b, ln2_w, ln2_b)
    wpr = np.zeros((MCHA * 128, C), np.float32)
    wpr[:M4] = proj_w
    wpr[M4] = proj_b
    return wq, wk, wv, wfc, np.ascontiguousarray(wpr)


def kernel(x, ln1_w, ln1_b, attn_w, attn_b, ln2_w, ln2_b, fc_w, fc_b, proj_w,
           proj_b, dims=None, n_cores=None, trace=False, dtype="fp16"):
    dims = dims or FULL_DIMS
    n_cores = n_cores if n_cores is not None else N_CORES
    B, T, C = dims["B"], dims["T"], dims["C"]
    TQ = T // 2
    x = np.asarray(x, np.float32)
    args = [np.asarray(a, np.float32) for a in
            (ln1_w, ln1_b, attn_w, attn_b, ln2_w, ln2_b, fc_w, fc_b, proj_w, proj_b)]
    wq, wk, wv, wfc, wpr = prep_weights(dims, *args)
    if dtype == "bf16":
        import ml_dtypes
        cast = lambda a: np.ascontiguousarray(a.astype(ml_dtypes.bfloat16))
    elif dtype == "fp16":
        cast = lambda a: np.ascontiguousarray(a.astype(np.float16))
    else:
        cast = lambda a: np.ascontiguousarray(a)
    wq, wk, wv, wfc, wpr = cast(wq), cast(wk), cast(wv), cast(wfc), cast(wpr)
    nc = _get_nc(dims, dtype)

    in_maps = []
    for c in range(n_cores):
        b, p = c // 2, c % 2
        xt = np.ascontiguousarray(x[b].T)
        if p == 1:
            xt = np.ascontiguousarray(np.concatenate([xt[:, TQ:], xt[:, :TQ]], axis=1))
        in_maps.append({
            "xT": cast(xt), "wq": wq, "wk": wk, "wv": wv, "wfc": wfc, "wpr": wpr,
            "pbias": np.array([[0.0 if p == 1 else -10000.0]], np.float32),
        })

    res = run_bass_kernel_spmd(nc, in_maps, core_ids=list(range(n_cores)), trace=trace)
    out = np.empty((B, T, C), np.float32)
    for c in range(n_cores):
        b, p = c // 2, c % 2
        out[b, p * TQ:(p + 1) * TQ, :] = res.results[c]["outT"].T
    if trace:
        return out, res
    return out
